# revision 1
# baseline (speedup 1.0000x reference)
"""Causal self-attention Trainium2 kernel.

Problem: B=8, T=1024, C=768, H=12 heads, D=64. fp32.
Sharding: data-parallel over batch — core b computes batch element b.

Per-core dataflow (everything transposed so softmax denominators and the
output projection both come out in the right orientation):

  xT [C, T]                          (host pre-transposed)
  qkT [2C, T] = w_qk.T @ x.T         (lhsT = w_qk blocks, rhs = xT)
  v   [T, C]  = x @ w_v              (lhsT = xT blocks, rhs = w_v)
  per head h, tq-512-chunk j:
    scoresT[tk, tq] = kT_h x qT_h    (lhsT = kT block [64,128], rhs = qT [64,512],
                                      two heads packed via PE row groups)
    expT = exp(0.125 * scoresT)      (ScalarE, no max subtraction; scores ~N(0,1))
    causal: skip blocks above diagonal, tril-mask diagonal 128x128 sub-blocks
    yT'[65, tq] = v_ext_h.T @ expT   (v_ext has a ones column -> row 64 = denom)
    copy yT' rows out of PSUM immediately (keeps PE fed); batch-normalize later:
    yT[0:64] *= bcast(1/denom)       (one batched reciprocal per tq-chunk)
  out [T, C] = yT.T @ w_proj         (lhsT = yT blocks, rhs = w_proj)

All matmul operands are float32r (FP22 multiply, fp32 accumulate, full
1 cycle/row streaming like bf16). Bias adds are compiled in only when the
corresponding bias is nonzero (the reference initializes them to zero).
"""

import numpy as np

import concourse.bass as bass
import concourse.bacc as bacc
import concourse.tile as tile
from concourse import mybir
from concourse.bass_utils import run_bass_kernel_spmd

N_CORES = 8
T = 1024
C = 768
H = 12
D = 64
P = 128
NT = T // P      # 8  t-chunks
NK = C // P      # 6  c-chunks (contraction)
NQC = (2 * C) // P  # 12 c'-chunks for q,k
F32 = mybir.dt.float32
F32R = mybir.dt.float32r
EXP = mybir.ActivationFunctionType.Exp


def build_kernel(qk_bias=False, v_bias=False, o_bias=False):
    nc = bacc.Bacc("TRN2", target_bir_lowering=False, debug=False,
                   num_devices=N_CORES)

    xT_d = nc.dram_tensor("xT", [C, T], F32R, kind="ExternalInput").ap()
    wqk_d = nc.dram_tensor("w_qk", [C, 2 * C], F32R, kind="ExternalInput").ap()
    wv_d = nc.dram_tensor("w_v", [C, C], F32R, kind="ExternalInput").ap()
    wp_d = nc.dram_tensor("w_proj", [C, C], F32R, kind="ExternalInput").ap()
    tril_d = nc.dram_tensor("tril", [P, P], F32, kind="ExternalInput").ap()
    ones_d = nc.dram_tensor("ones12", [P, H], F32R, kind="ExternalInput").ap()
    ones64_d = nc.dram_tensor("ones64", [P, D], F32R, kind="ExternalInput").ap()
    if qk_bias:
        bqk_d = nc.dram_tensor("b_qk_cols", [P, NQC], F32,
                               kind="ExternalInput").ap()
    if v_bias:
        bv_d = nc.dram_tensor("bias_v_b", [P, C], F32,
                              kind="ExternalInput").ap()
    if o_bias:
        bo_d = nc.dram_tensor("bias_o_b", [P, C], F32,
                              kind="ExternalInput").ap()
    out_d = nc.dram_tensor("out", [T, C], F32, kind="ExternalOutput").ap()

    with tile.TileContext(nc) as tc:
        with tc.tile_pool(name="persist", bufs=1) as pp_sb:
            qkT = [pp_sb.tile([P, T], F32R, tag=f"qkT{j}", name=f"qkT{j}")
                   for j in range(NQC)]
            v_ext = [pp_sb.tile([P, H, D + 1], F32R, tag=f"vext{i}",
                                name=f"vext{i}") for i in range(NT)]
            yT = [pp_sb.tile([P, T], F32R, tag=f"yT{k}", name=f"yT{k}")
                  for k in range(NK)]
            tril = pp_sb.tile([P, P], F32, tag="tril")
            nc.sync.dma_start(tril[:], tril_d[:])
            ones64 = pp_sb.tile([P, D], F32R, tag="ones64")
            nc.sync.dma_start(ones64[:], ones64_d[:])
            if qk_bias:
                bqk = pp_sb.tile([P, NQC], F32, tag="bqk")
                nc.sync.dma_start(bqk[:], bqk_d[:])
            if v_bias:
                bv = pp_sb.tile([P, C], F32, tag="bv")
                nc.sync.dma_start(bv[:], bv_d[:])
            if o_bias:
                bo = pp_sb.tile([P, C], F32, tag="bo")
                nc.sync.dma_start(bo[:], bo_d[:])

            # ---------------- phase 1: QKV projections ----------------
            # one PSUM pool for the whole kernel: "small" (1-bank) slots are
            # shared by qk-proj, out-proj and the reciprocal broadcasts;
            # "big" (2-bank) slots by the v-proj and attention scores. This
            # avoids a phase-boundary bank handoff that would serialize the
            # qkv and attention phases.
            aps = tc.alloc_tile_pool(name="ps", bufs=2, space="PSUM")
            with tc.tile_pool(name="qkv_sb", bufs=1) as qs:
                xT = [qs.tile([P, T], F32R, tag=f"xT{k}", name=f"xT{k}")
                      for k in range(NK)]
                wqk = [qs.tile([P, 2 * C], F32R, tag=f"wqk{k}", name=f"wqk{k}")
                       for k in range(NK)]
                wv = [qs.tile([P, C], F32R, tag=f"wv{k}", name=f"wv{k}")
                      for k in range(NK)]
                for k in range(NK):
                    nc.sync.dma_start(xT[k][:], xT_d[P * k:P * (k + 1), :])
                    nc.sync.dma_start(wqk[k][:], wqk_d[P * k:P * (k + 1), :])
                for k in range(NK):
                    nc.sync.dma_start(wv[k][:], wv_d[P * k:P * (k + 1), :])

                def qk_chunk(j):
                    for t2 in range(2):
                        ps = aps.tile([P, 512], F32, tag="small",
                                      name="ps_qk")
                        for k in range(NK):
                            nc.tensor.matmul(
                                ps[:],
                                wqk[k][:, P * j:P * (j + 1)],
                                xT[k][:, 512 * t2:512 * (t2 + 1)],
                                start=(k == 0), stop=(k == NK - 1))
                        dst = qkT[j][:, 512 * t2:512 * (t2 + 1)]
                        if qk_bias:
                            nc.vector.tensor_scalar_add(
                                out=dst, in0=ps[:], scalar1=bqk[:, j:j + 1])
                        else:
                            nc.vector.tensor_copy(dst, ps[:])

                # head pair 0 first so attention can start early
                qk_chunk(0)
                qk_chunk(6)
                # v (normal orientation) + ones column (+ bias)
                for i in range(NT):
                    ps = aps.tile([P, 1024], F32, tag="big", name="ps_v")
                    for k in range(NK):
                        lhsT = xT[k][:, P * i:P * (i + 1)]
                        nc.tensor.matmul(ps[:, 0:512], lhsT,
                                         wv[k][:, 0:512],
                                         start=(k == 0), stop=(k == NK - 1))
                        nc.tensor.matmul(ps[:, 512:768], lhsT,
                                         wv[k][:, 512:768],
                                         start=(k == 0), stop=(k == NK - 1))
                    ps = ps[:, 0:C]
                    nc.sync.dma_start(
                        v_ext[i][:, :, D:D + 1],
                        ones_d.rearrange("p (f o) -> p f o", o=1))
                    ps3 = ps.rearrange("p (h d) -> p h d", h=H)
                    if v_bias:
                        nc.vector.tensor_add(
                            out=v_ext[i][:, :, 0:D], in0=ps3,
                            in1=bv.rearrange("p (h d) -> p h d", h=H))
                    else:
                        nc.vector.tensor_copy(v_ext[i][:, :, 0:D], ps3)
                for g in range(1, 6):
                    qk_chunk(g)
                    qk_chunk(6 + g)

            # ------------- phase 2+3: attention, projection -------------
            with tc.tile_pool(name="attn_sb", bufs=4) as asb, \
                 tc.tile_pool(name="attn_sb2", bufs=2) as asb2, \
                 tc.tile_pool(name="proj_sb", bufs=1) as psb, \
                 tc.tile_pool(name="out_sb", bufs=3) as osb:
                wproj = [psb.tile([P, C], F32R, tag=f"wp{k}", name=f"wp{k}")
                         for k in range(NK)]
                for k in range(NK):
                    nc.sync.dma_start(wproj[k][:], wp_d[P * k:P * (k + 1), :])

                def attn_chunk(j2, filler=None):
                    tq0 = 512 * j2
                    n_tk = 4 * (j2 + 1)
                    # dens for heads 4m+r live at partition 32r of den_t[m]
                    # (SBUF accesses must start at 32-aligned partitions)
                    den_t = [asb2.tile([P, 512], F32, tag=f"den{m}",
                                       name=f"den{m}") for m in range(3)]
                    rec_t = [asb2.tile([P, 512], F32, tag=f"rec{m}",
                                       name=f"rec{m}") for m in range(3)]
                    recr_t = [asb2.tile([P, 512], F32R, tag=f"recr{m}",
                                        name=f"recr{m}") for m in range(3)]
                    for g in range(6):
                        if filler is not None and g >= 2:
                            filler(g - 2)
                        ps_y = [aps.tile([D + 1, 512], F32, tag="ps_y",
                                         name="ps_y") for _ in range(2)]
                        for c2 in range(n_tk // 2):
                            ps_s = []
                            ex = []
                            offs = [max(0, P * (2 * c2 + s) - tq0)
                                    for s in range(2)]
                            for hh in range(2):
                                po = D * hh
                                t_s = aps.tile([P, 1024], F32, tag="big",
                                               name="ps_s")
                                ps_s.append(t_s)
                                for s in range(2):
                                    c = 2 * c2 + s
                                    off = offs[s]
                                    nc.tensor.matmul(
                                        t_s[:, 512 * s + off:512 * (s + 1)],
                                        qkT[6 + g][po:po + D,
                                                   P * c:P * (c + 1)],
                                        qkT[g][po:po + D, tq0 + off:tq0 + 512],
                                        start=True, stop=True)
                            for hh in range(2):
                                t_s = ps_s[hh]
                                t_e = asb.tile([P, 1024], F32R, tag="exp",
                                               name="exp")
                                ex.append(t_e)
                                nc.scalar.activation(
                                    out=t_e[:], in_=t_s[:], func=EXP,
                                    scale=0.125)
                                if offs[0] > 0 or offs[1] > 0:
                                    # both chunks straddle the diagonal:
                                    # mask the two 128-col sub-blocks in one
                                    # strided TT against tril
                                    o0 = offs[0]
                                    sl = bass.AP(
                                        tensor=t_e.tensor,
                                        offset=t_e.offset + o0,
                                        ap=[t_e.ap[0], [512 + P, 2], [1, P]])
                                    trb = bass.AP(
                                        tensor=tril.tensor,
                                        offset=tril.offset,
                                        ap=[tril.ap[0], [0, 2], [1, P]])
                                    nc.vector.tensor_mul(
                                        out=sl, in0=sl, in1=trb)
                                h = 2 * g + hh
                                for s in range(2):
                                    c = 2 * c2 + s
                                    off = offs[s]
                                    nc.tensor.matmul(
                                        ps_y[hh][:, off:512],
                                        v_ext[c][:, h, :],
                                        t_e[:, 512 * s + off:512 * (s + 1)],
                                        start=(c == 0), stop=(c == n_tk - 1))
                        # drain PSUM fast; normalization happens later
                        for hh in range(2):
                            h = 2 * g + hh
                            nc.vector.tensor_copy(
                                den_t[h // 4][32 * (h % 4):32 * (h % 4) + 1, :],
                                ps_y[hh][D:D + 1, :])
                            nc.vector.tensor_copy(
                                yT[g][D * hh:D * (hh + 1), tq0:tq0 + 512],
                                ps_y[hh][0:D, :])
                        if g % 2 == 1:
                            m = (g - 1) // 2
                            nc.vector.reciprocal_approx_fast(
                                out=rec_t[m][:], in_=den_t[m][:])
                            nc.vector.tensor_copy(
                                recr_t[m][:], rec_t[m][:])
                    # broadcast each head's reciprocal row across partitions
                    # on the PE (ones-column matmul into PSUM), then multiply
                    # in place (TT with the PSUM operand sidesteps the
                    # equal-base-partition rule for SBUF/SBUF TT pairs)
                    for g in range(6):
                        for hh in range(2):
                            h = 2 * g + hh
                            r = 32 * (h % 4)
                            bc_ps = aps.tile([D, 512], F32, tag="small",
                                             name="bc_ps")
                            nc.tensor.matmul(
                                bc_ps[:],
                                ones64[r:r + 1, :],
                                recr_t[h // 4][r:r + 1, :],
                                start=True, stop=True,
                                tile_position=(r, 0) if r == 96 else None)
                            dst = yT[g][D * hh:D * (hh + 1), tq0:tq0 + 512]
                            nc.vector.tensor_mul(out=dst, in0=bc_ps[:],
                                                 in1=dst)

                def proj_chunk(i):
                    o_t = osb.tile([P, C], F32, tag="out", name="outt")
                    for n in range(2):
                        ps_o = aps.tile([P, 384], F32, tag="small",
                                        name="ps_o")
                        for k in range(NK):
                            nc.tensor.matmul(
                                ps_o[:],
                                yT[k][:, P * i:P * (i + 1)],
                                wproj[k][:, 384 * n:384 * (n + 1)],
                                start=(k == 0), stop=(k == NK - 1))
                        dst = o_t[:, 384 * n:384 * (n + 1)]
                        if o_bias:
                            nc.vector.tensor_add(
                                out=dst, in0=ps_o[:],
                                in1=bo[:, 384 * n:384 * (n + 1)])
                        else:
                            nc.vector.tensor_copy(dst, ps_o[:])
                    nc.sync.dma_start(out_d[P * i:P * (i + 1), :], o_t[:])

                attn_chunk(0)
                attn_chunk(1, filler=proj_chunk)
                for i in range(4, 8):
                    proj_chunk(i)
            aps.release()

    nc.compile()
    return nc


_NC_CACHE = {}


def _get_nc(qk_bias, v_bias, o_bias):
    key = (qk_bias, v_bias, o_bias)
    if key not in _NC_CACHE:
        _NC_CACHE[key] = build_kernel(*key)
    return _NC_CACHE[key]


def make_in_maps(x, w_attn, b_attn, w_proj, b_proj, qk_bias, v_bias, o_bias):
    w_qk = np.ascontiguousarray(w_attn[:, :2 * C])
    w_v = np.ascontiguousarray(w_attn[:, 2 * C:])
    # tril[tk, l] = 1 iff l >= tk  (keep query-pos >= key-pos)
    tril = np.triu(np.ones((P, P), dtype=np.float32))

    shared = {
        "w_qk": w_qk, "w_v": w_v, "w_proj": np.ascontiguousarray(w_proj),
        "tril": tril,
        "ones12": np.ones((P, H), dtype=np.float32),
        "ones64": np.ones((P, D), dtype=np.float32),
    }
    if qk_bias:
        shared["b_qk_cols"] = np.ascontiguousarray(
            b_attn[:2 * C].reshape(NQC, P).T)
    if v_bias:
        shared["bias_v_b"] = np.ascontiguousarray(
            np.broadcast_to(b_attn[2 * C:], (P, C)))
    if o_bias:
        shared["bias_o_b"] = np.ascontiguousarray(
            np.broadcast_to(b_proj, (P, C)))
    in_maps = []
    for b in range(N_CORES):
        m = dict(shared)
        m["xT"] = np.ascontiguousarray(x[b].T)
        in_maps.append(m)
    return in_maps


def run(x, w_attn, b_attn, w_proj, b_proj, **spmd_kwargs):
    x = np.asarray(x, dtype=np.float32)
    w_attn = np.asarray(w_attn, dtype=np.float32)
    b_attn = np.asarray(b_attn, dtype=np.float32)
    w_proj = np.asarray(w_proj, dtype=np.float32)
    b_proj = np.asarray(b_proj, dtype=np.float32)
    qk_bias = bool(np.any(b_attn[:2 * C]))
    v_bias = bool(np.any(b_attn[2 * C:]))
    o_bias = bool(np.any(b_proj))
    nc = _get_nc(qk_bias, v_bias, o_bias)
    in_maps = make_in_maps(x, w_attn, b_attn, w_proj, b_proj,
                           qk_bias, v_bias, o_bias)
    res = run_bass_kernel_spmd(nc, in_maps, core_ids=list(range(N_CORES)),
                               **spmd_kwargs)
    out = np.stack([res.results[b]["out"] for b in range(N_CORES)], axis=0)
    return out.astype(np.float32), res


def kernel(x, w_attn, b_attn, w_proj, b_proj):
    out, _ = run(x, w_attn, b_attn, w_proj, b_proj)
    return out



# revision 28
# speedup vs baseline: 1.2479x; 1.2479x over previous
"""Causal self-attention Trainium2 kernel (v2, bf16).

Problem: B=8, T=1024, C=768, H=12 heads, D=64. fp32 in/out.
Sharding: data-parallel over batch -- core b computes batch element b.

All SBUF operands are bf16 (inputs cast on host; rel err ~6e-3 vs the
f32 reference, gate is 2e-2). PSUM stays f32. Output DMA'd as bf16 and
upcast on host.

Per-core dataflow (transposed so softmax denominators and the output
projection both come out in the right orientation):

  xT [C, T]                       (host pre-transposed, chunk-major)
  qkT[j] [128, T] = w_j.T @ xT    j<6: q pair j, j>=6: k pair j-6
  v   [T, C]  = x @ w_v           (+ ones column per head -> denom row)
  per head-pair g, tq-512 window j2, tile (2 tk-chunks x 1 head):
    scoresT[tk, tq] = kT_h x qT_h   (two heads in PE row groups)
    mask: matmul-accumulate -30000*triu(1) onto diagonal 128-blocks
    expT = exp(0.125 * scoresT)     (ScalarE, valid columns only)
    yT'[65, tq] += v_ext.T @ expT   (row 64 = denominator)
  normalize: recip(den) -> PE ones-broadcast -> DVE mul in place
  out [T, C] = yT.T @ w_proj      (bf16 out, upcast on host)

QK/V/out-proj matmuls are interleaved as PE "filler" between attention
tiles so the PE stays busy while ScalarE runs exp (exp is the attention
bottleneck: ~72 calls x ~1.07us).
"""

from collections import deque

import numpy as np
import ml_dtypes

import concourse.bass as bass
import concourse.bacc as bacc
import concourse.tile as tile
from concourse import mybir
from concourse.bass_utils import run_bass_kernel_spmd

N_CORES = 8
T = 1024
C = 768
H = 12
D = 64
P = 128
NT = T // P       # 8  t-chunks
NK = C // P       # 6  c-chunks (contraction)
F32 = mybir.dt.float32
F32R = mybir.dt.float32r
BF = mybir.dt.bfloat16
EXP = mybir.ActivationFunctionType.Exp
BF_NP = ml_dtypes.bfloat16
NEG = -30000.0    # causal mask additive constant (0.125*NEG << exp underflow)
DEFER_DRAIN = True


def build_kernel(qk_bias=False, v_bias=False, o_bias=False, debug_dump=False):
    nc = bacc.Bacc("TRN2", target_bir_lowering=False, debug=False,
                   num_devices=N_CORES)
    if debug_dump:
        qkT_dump = nc.dram_tensor("qkT_dump", [12, P, T], BF,
                                  kind="ExternalOutput").ap()
        yT_dump = nc.dram_tensor("yT_dump", [NK, P, T], BF,
                                 kind="ExternalOutput").ap()
        vx_dump = nc.dram_tensor("vx_dump", [NT, P, H * (D + 1)], BF,
                                 kind="ExternalOutput").ap()

    # host layouts are chunk-major: [P, NK, cols] so each is one big DMA
    xT_d = nc.dram_tensor("xT6", [P, NK, T], BF, kind="ExternalInput").ap()
    wqkA_d = nc.dram_tensor("wqkA", [P, NK, 256], BF,
                            kind="ExternalInput").ap()   # pair-0 q|k cols
    wqkB_d = nc.dram_tensor("wqkB", [P, NK, 5, 256], BF,
                            kind="ExternalInput").ap()   # pairs 1-5 q|k
    wv_d = nc.dram_tensor("wv6", [P, NK, C], BF, kind="ExternalInput").ap()
    wp_d = nc.dram_tensor("wp6", [P, NK, C], BF, kind="ExternalInput").ap()
    cb_d = nc.dram_tensor("cb", [P, 2, P], BF,
                          kind="ExternalInput").ap()     # maskT | I128
    ind_d = nc.dram_tensor("ind2", [1, 2 * P], F32R, kind="ExternalInput").ap()
    if qk_bias:
        bqk_d = nc.dram_tensor("bqk", [P, 12], F32, kind="ExternalInput").ap()
    if v_bias:
        bv_d = nc.dram_tensor("bv", [P, C], BF, kind="ExternalInput").ap()
    if o_bias:
        bo_d = nc.dram_tensor("bo", [P, C], BF, kind="ExternalInput").ap()
    out_d = nc.dram_tensor("out", [T, C], BF, kind="ExternalOutput").ap()

    with tile.TileContext(nc) as tc:
        with tc.tile_pool(name="persist", bufs=1) as pp:
            qkT = [pp.tile([P, T], BF, tag=f"qkT{j}", name=f"qkT{j}")
                   for j in range(12)]
            v_ext = [pp.tile([P, H, D + 1], BF, tag=f"vext{i}",
                             name=f"vext{i}") for i in range(NT)]
            yT = [pp.tile([P, T], BF, tag=f"yT{k}", name=f"yT{k}")
                  for k in range(NK)]
            xT = pp.tile([P, NK, T], BF, tag="xT")
            wqkA = pp.tile([P, NK, 256], BF, tag="wqkA")
            wqkB = pp.tile([P, NK, 5, 256], BF, tag="wqkB")
            wv = pp.tile([P, NK, C], BF, tag="wv")
            wp = pp.tile([P, NK, C], BF, tag="wp")
            cb = pp.tile([P, 2, P], BF, tag="cb")
            ind2 = pp.tile([1, 2 * P], F32R, tag="ind2")

            # DMA order = availability order for the PE schedule
            nc.sync.dma_start(xT[:], xT_d[:])
            nc.sync.dma_start(wqkA[:], wqkA_d[:])
            nc.sync.dma_start(cb[:], cb_d[:])
            nc.sync.dma_start(ind2[:], ind_d[:])
            nc.sync.dma_start(wv[:], wv_d[:])
            nc.sync.dma_start(wqkB[:], wqkB_d[:])
            nc.sync.dma_start(wp[:], wp_d[:])
            if qk_bias:
                bqk = pp.tile([P, 12], F32, tag="bqk")
                nc.sync.dma_start(bqk[:], bqk_d[:])
            if v_bias:
                bv = pp.tile([P, C], BF, tag="bv")
                nc.sync.dma_start(bv[:], bv_d[:])
            if o_bias:
                bo = pp.tile([P, C], BF, tag="bo")
                nc.sync.dma_start(bo[:], bo_d[:])
            maskT = cb[:, 0, :]
            ident = cb[:, 1, :]

            # ones column of v_ext (denominator trick), set once
            for i in range(NT):
                nc.gpsimd.memset(v_ext[i][:, :, D:D + 1], 1.0)

            ps = tc.alloc_tile_pool(name="ps", bufs=2, space="PSUM")
            # psA: attention score tiles [P,1024] (2 banks x2 = 4 banks)
            # psB: filler qk/v/proj + recip-broadcast (1 bank x2 = 2)
            # psY: per-head y accumulators [D+1,512]  (1 bank x2 = 2)

            with tc.tile_pool(name="esb", bufs=6) as esb, \
                 tc.tile_pool(name="osb", bufs=3) as osb, \
                 tc.tile_pool(name="rsb", bufs=4) as rsb:

                def wq_slice(j, k):
                    # lhsT [128, 128] for qk chunk j (j<6: q pair j else k)
                    g, qk = j % 6, j // 6
                    if g == 0:
                        return wqkA[:, k, 128 * qk:128 * (qk + 1)]
                    return wqkB[:, k, g - 1, 128 * qk:128 * (qk + 1)]

                def qk_half(j, t2):
                    b = ps.tile([P, 512], F32, tag="psB", name="psB")
                    for k in range(NK):
                        nc.tensor.matmul(
                            b[:], wq_slice(j, k),
                            xT[:, k, 512 * t2:512 * (t2 + 1)],
                            start=(k == 0), stop=(k == NK - 1))
                    dst = qkT[j][:, 512 * t2:512 * (t2 + 1)]
                    if qk_bias:
                        nc.vector.tensor_scalar_add(
                            out=dst, in0=b[:], scalar1=bqk[:, j:j + 1])
                    else:
                        nc.vector.tensor_copy(dst, b[:])

                def qk_gen(j):
                    for t2 in range(2):
                        qk_half(j, t2)
                        yield

                def v_gen(i):
                    # v chunk i: out rows t in [128i,128(i+1)), cols 0:768
                    for n, (c0, cw, h0, hn) in enumerate(
                            ((0, 512, 0, 8), (512, 256, 8, 4))):
                        b = ps.tile([P, 512], F32, tag="psB", name="psB")
                        for k in range(NK):
                            nc.tensor.matmul(
                                b[:, 0:cw],
                                xT[:, k, P * i:P * (i + 1)],
                                wv[:, k, c0:c0 + cw],
                                start=(k == 0), stop=(k == NK - 1))
                        src = b[:, 0:cw].rearrange("p (h d) -> p h d", d=D)
                        dst = v_ext[i][:, h0:h0 + hn, 0:D]
                        if v_bias:
                            nc.vector.tensor_add(
                                out=dst, in0=src,
                                in1=bv[:, c0:c0 + cw].rearrange(
                                    "p (h d) -> p h d", d=D))
                        else:
                            nc.vector.tensor_copy(dst, src)
                        yield

                def proj_gen(i):
                    o_t = osb.tile([P, C], BF, tag="out", name="outt")
                    for n in range(2):
                        b = ps.tile([P, 512], F32, tag="psB", name="psB")
                        for k in range(NK):
                            nc.tensor.matmul(
                                b[:, 0:384],
                                yT[k][:, P * i:P * (i + 1)],
                                wp[:, k, 384 * n:384 * (n + 1)],
                                start=(k == 0), stop=(k == NK - 1))
                        dst = o_t[:, 384 * n:384 * (n + 1)]
                        if o_bias:
                            nc.vector.tensor_add(out=dst, in0=b[:, 0:384],
                                                 in1=bo[:, 384 * n:384 * (n + 1)])
                        else:
                            nc.vector.tensor_copy(dst, b[:, 0:384])
                        yield
                    nc.sync.dma_start(out_d[P * i:P * (i + 1), :], o_t[:])

                filler = deque()

                def fstep(n=1):
                    for _ in range(n):
                        while filler:
                            try:
                                next(filler[0])
                                break
                            except StopIteration:
                                filler.popleft()

                pending_drain = [None]

                def window(g, j2, fsteps=1, skip_fstep_until=0):
                    tq0 = 512 * j2
                    n_tk = 4 * (j2 + 1)
                    M = n_tk // 2
                    psy = [ps.tile([D + 1, 512], F32, tag="psY", name="psY")
                           for _ in range(2)]
                    tes = {}

                    def emit_av(m):
                        for hh in range(2):
                            te, offs = tes[(m, hh)]
                            for s in range(2):
                                c = 2 * m + s
                                nc.tensor.matmul(
                                    psy[hh][:, offs[s]:512],
                                    v_ext[c][:, 2 * g + hh, :],
                                    te[:, 512 * s + offs[s]:512 * (s + 1)],
                                    start=(c == 0), stop=(c == n_tk - 1))

                    pend = deque()
                    for m in range(M):
                        c0, c1 = 2 * m, 2 * m + 1
                        offs = [max(0, P * c - tq0) for c in (c0, c1)]
                        diag = offs[0] > 0 or offs[1] > 0 or (j2 == 0 and m == 0)
                        for hh in range(2):
                            sp = ps.tile([P, 1024], F32, tag="psA", name="psA")
                            if offs[1] > 0:
                                # the flat exp read spans [offs[0]:1024] and
                                # crosses into bank 1 at col 512; init the
                                # [512:512+offs[1]] hole (tq<tk for chunk c1;
                                # finite, never consumed by AV)
                                for o in range(0, offs[1], P):
                                    nc.tensor.matmul(
                                        sp[:, 512 + o:512 + o + P],
                                        maskT, ident,
                                        start=True, stop=True)
                            for s, c in ((0, c0), (1, c1)):
                                nc.tensor.matmul(
                                    sp[:, 512 * s + offs[s]:512 * (s + 1)],
                                    qkT[6 + g][D * hh:D * (hh + 1),
                                               P * c:P * (c + 1)],
                                    qkT[g][D * hh:D * (hh + 1),
                                           tq0 + offs[s]:tq0 + 512],
                                    start=True, stop=True)
                                if diag:
                                    o = 512 * s + offs[s]
                                    nc.tensor.matmul(
                                        sp[:, o:o + P], maskT, ident,
                                        start=False, stop=True,
                                        skip_group_check=True)
                            te = esb.tile([P, 1024], BF, tag="exp",
                                          name="exp")
                            nc.scalar.activation(
                                out=te[:, offs[0]:1024],
                                in_=sp[:, offs[0]:1024], func=EXP,
                                scale=0.125)
                            tes[(m, hh)] = (te, offs)
                        if m == 1 and pending_drain[0] is not None:
                            pending_drain[0]()
                            pending_drain[0] = None
                        pend.append(m)
                        if len(pend) > 1:
                            emit_av(pend.popleft())
                        if m >= skip_fstep_until:
                            fstep(fsteps)
                    while pend:
                        emit_av(pend.popleft())

                    # drain: evacuate y + denominators; ps_y freed here
                    recrs = []
                    for hh in range(2):
                        nc.vector.tensor_copy(
                            yT[g][D * hh:D * (hh + 1), tq0:tq0 + 512],
                            psy[hh][0:D, :])
                        den = rsb.tile([1, 512], F32, tag="den", name="den")
                        nc.vector.tensor_copy(den[:], psy[hh][D:D + 1, :])
                        rec = rsb.tile([1, 512], F32, tag="rec", name="rec")
                        nc.vector.reciprocal_approx_fast(
                            out=rec[:], in_=den[:])
                        recr = rsb.tile([1, 512], F32R, tag="recr",
                                        name="recr")
                        nc.vector.tensor_copy(recr[:], rec[:])
                        recrs.append(recr)

                    def deferred():
                        # broadcast rec row hh across partitions 64hh..64hh+63
                        # via two accumulating K=1 indicator matmuls
                        bc = ps.tile([P, 512], F32, tag="psB", name="psB")
                        for hh in range(2):
                            nc.tensor.matmul(
                                bc[:], ind2[:, P * hh:P * (hh + 1)],
                                recrs[hh][:],
                                start=(hh == 0), stop=(hh == 1))
                        for hh in range(2):
                            dst = yT[g][D * hh:D * (hh + 1), tq0:tq0 + 512]
                            nc.vector.tensor_mul(
                                out=dst, in0=bc[D * hh:D * (hh + 1), :],
                                in1=dst)
                    if DEFER_DRAIN:
                        pending_drain[0] = deferred
                    else:
                        deferred()

                # ---- schedule ----
                for t2 in range(2):
                    qk_half(0, t2)
                    qk_half(6, t2)
                for i in range(4):
                    for _ in v_gen(i):
                        pass
                filler.extend([v_gen(4), v_gen(5), v_gen(6), v_gen(7),
                               qk_gen(1), qk_gen(7)])
                window(0, 0, fsteps=2)
                window(0, 1, fsteps=2)
                for g in range(1, 6):
                    if g < 5:
                        filler.extend([qk_gen(g + 1), qk_gen(g + 7)])
                        window(g, 0, fsteps=1)
                        window(g, 1, fsteps=1)
                    else:
                        window(g, 0, fsteps=1)
                        # proj 0..3 read yT[5][:, 0:512], written by pair5-w0's
                        # drain -- flushed at w1's m==1; gate fillers past it
                        filler.extend([proj_gen(0), proj_gen(1),
                                       proj_gen(2), proj_gen(3)])
                        window(g, 1, fsteps=2, skip_fstep_until=1)
                if pending_drain[0] is not None:
                    pending_drain[0]()
                    pending_drain[0] = None
                fstep(24)   # flush remaining fillers
                for i in range(4, 8):
                    for _ in proj_gen(i):
                        pass
                if debug_dump:
                    for j in range(12):
                        nc.sync.dma_start(qkT_dump[j], qkT[j][:])
                    for k in range(NK):
                        nc.sync.dma_start(yT_dump[k], yT[k][:])
                    for i in range(NT):
                        nc.sync.dma_start(
                            vx_dump[i],
                            v_ext[i][:].rearrange("p h d -> p (h d)"))
            ps.release()

    nc.compile()
    return nc


_NC_CACHE = {}


def _get_nc(qk_bias, v_bias, o_bias):
    key = (qk_bias, v_bias, o_bias)
    if key not in _NC_CACHE:
        _NC_CACHE[key] = build_kernel(*key)
    return _NC_CACHE[key]


def _chunk_major(a):
    # [C, cols] -> [P, NK, cols]: row chunk k lands at [:, k, :]
    cols = a.shape[1]
    return np.ascontiguousarray(
        a.reshape(NK, P, cols).transpose(1, 0, 2)).astype(BF_NP)


def make_in_maps(x, w_attn, b_attn, w_proj, b_proj, qk_bias, v_bias, o_bias):
    wq = w_attn[:, 0:C]
    wk = w_attn[:, C:2 * C]
    wv = w_attn[:, 2 * C:]
    # pair g columns: q block [128g:128g+128], k block likewise
    pairs = [np.concatenate([wq[:, P * g:P * (g + 1)],
                             wk[:, P * g:P * (g + 1)]], axis=1)
             for g in range(6)]
    wqkA = _chunk_major(pairs[0])
    wqkB = _chunk_major(np.concatenate(pairs[1:], axis=1)).reshape(
        P, NK, 5, 256)
    maskT = (NEG * np.triu(np.ones((P, P), np.float32), 1)).astype(BF_NP)
    ident = np.eye(P, dtype=np.float32).astype(BF_NP)
    cb = np.stack([maskT, ident], axis=1)  # [P, 2, P]

    # ind2[0, 128*hh + m] = 1 iff output partition m belongs to head hh
    ind2 = np.zeros((1, 2 * P), dtype=np.float32)
    ind2[0, 0:D] = 1.0
    ind2[0, P + D:P + 2 * D] = 1.0
    shared = {
        "wqkA": wqkA, "wqkB": wqkB,
        "wv6": _chunk_major(wv), "wp6": _chunk_major(w_proj),
        "cb": cb,
        "ind2": ind2,
    }
    if qk_bias:
        # bias for qkT[j] partitions: j<6 -> q pair j, j>=6 -> k pair j-6
        bq = b_attn[0:C].reshape(6, P).T          # [P, 6]
        bk = b_attn[C:2 * C].reshape(6, P).T
        shared["bqk"] = np.ascontiguousarray(
            np.concatenate([bq, bk], axis=1)).astype(np.float32)
    if v_bias:
        shared["bv"] = np.broadcast_to(
            b_attn[2 * C:], (P, C)).astype(BF_NP)
    if o_bias:
        shared["bo"] = np.broadcast_to(b_proj, (P, C)).astype(BF_NP)
    in_maps = []
    for b in range(N_CORES):
        m = dict(shared)
        m["xT6"] = _chunk_major(np.ascontiguousarray(x[b].T))
        in_maps.append(m)
    return in_maps


def run(x, w_attn, b_attn, w_proj, b_proj, **spmd_kwargs):
    x = np.asarray(x, dtype=np.float32)
    w_attn = np.asarray(w_attn, dtype=np.float32)
    b_attn = np.asarray(b_attn, dtype=np.float32)
    w_proj = np.asarray(w_proj, dtype=np.float32)
    b_proj = np.asarray(b_proj, dtype=np.float32)
    qk_bias = bool(np.any(b_attn[:2 * C]))
    v_bias = bool(np.any(b_attn[2 * C:]))
    o_bias = bool(np.any(b_proj))
    nc = _get_nc(qk_bias, v_bias, o_bias)
    in_maps = make_in_maps(x, w_attn, b_attn, w_proj, b_proj,
                           qk_bias, v_bias, o_bias)
    res = run_bass_kernel_spmd(nc, in_maps, core_ids=list(range(N_CORES)),
                               **spmd_kwargs)
    out = np.stack([res.results[b]["out"] for b in range(N_CORES)], axis=0)
    return out.astype(np.float32), res


def kernel(x, w_attn, b_attn, w_proj, b_proj):
    out, _ = run(x, w_attn, b_attn, w_proj, b_proj)
    return out
